# revision 1
# baseline (speedup 1.0000x reference)
"""BasisResidualFFN Trainium2 kernel.

Math (per token t):
  recipe_soft = softmax(neuron_recipe, axis=-1)                 [64, 16]
  tr[t, :]    = sum_k w[t,k] * recipe_soft[idx[t,k], :]         [16]
  Y[t, (n,r)] = sum_d x[t,d] * basis_A[n,d,r]
  h[t, r]     = sum_n tr[t,n] * Y[t,(n,r)]
  delta[t, d] = sum_{n,r} basis_A[n,d,r] * tr[t,n] * h[t,r]
  out         = gelu((x + alpha*delta) @ w_up + b_up) @ w_down + b_down

Distribution: pure data parallel. B*S = 4096 tokens sharded 512/core
across 8 NeuronCores; all weights replicated. Everything on device is
computed feature-major (features on partitions, tokens on the free
axis, 512 tokens per matmul) so no on-device activation transposes are
needed anywhere in the FFN; x arrives pre-transposed from the host and
the output is un-transposed on the host.

Precision: everything runs bf16 (basis-path errors enter only through
alpha*delta with alpha ~ 0.1; the residual x is bf16-rounded, which
costs ~2^-9 relative — far inside the error budget).

Schedule: the PE must never idle (HAM re-throttles to 1.2 GHz after
~3.4us of idle). Warm-up matmuls on a memset tile start at user-code
time zero with no DMA dependency; the YT matmuls (independent of
routing) are interleaved with the routing matmuls so the tensor queue
stays dense while the Vector engine runs the routing scatter (a single
fused chain over all 4 token-tiles; GpSimd has no usable tensor ops in
this toolchain, so it only memsets and triggers DMAs). The per-token
recipe replication is restructured as RepR[i] = (recs @ SEL_i)^T @ S^T
so the small M_i = (recs @ SEL_i)^T matmuls happen early, off the
critical path, instead of a serial rt -> recipeT -> rp chain. alpha is
folded into A2 on the host. The second half of the residual is formed
by preloading x into PSUM with an identity matmul so its drains
(Scalar) don't occupy Vector, which is the scarce engine.
"""

import numpy as np

import concourse.bass as bass
import concourse.mybir as mybir
import concourse.tile as tile
from concourse import bacc
from concourse.bass import ts
from concourse.bass_utils import run_bass_kernel_spmd

P = 128
NCORES = 8
T = 512            # tokens per core
D = 1024
DFF = 4096
NB = 16            # n_basis
R = 32             # rank
NN = 64            # n_neurons
K = 8              # top-k
DC = D // P        # 8 contraction chunks over d
FT = DFF // P      # 32 ff tiles
DT = D // P        # 8 output d tiles
NRT = (NB * R) // P  # 4 (n,r) tiles
TT = T // P        # 4 token tiles per core

# const blob column layouts (bf16 blob / f32 blob)
BR_SEL, BR_TREP, BR_QRED, BR_IOTA, BR_ID, BR_I512, BR_W = 0, 512, 640, 672, 736, 864, 1376
BF_ID, BF_BU, BF_BD, BF_AL, BF_REC, BF_W = 0, 128, 160, 168, 169, 192

F32 = mybir.dt.float32
BF16 = mybir.dt.bfloat16

NWARM = 10

_BUILT = [None]


def _build_nc():
    nc = bacc.Bacc(None, target_bir_lowering=False)

    xtb_d = nc.dram_tensor("xtb", [P, DC, T], BF16, kind="ExternalInput")
    idxw_d = nc.dram_tensor("idxw", [P, TT, 2 * K], BF16, kind="ExternalInput")
    blobr_d = nc.dram_tensor("blobr", [P, BR_W], BF16, kind="ExternalInput")
    blobf_d = nc.dram_tensor("blobf", [P, BF_W], F32, kind="ExternalInput")
    a1_d = nc.dram_tensor("a1", [P, DC, NB * R], BF16, kind="ExternalInput")
    a2_d = nc.dram_tensor("a2", [P, NRT, D], BF16, kind="ExternalInput")
    wu_d = nc.dram_tensor("wu", [FT // 2, P, 2, DC, P], BF16, kind="ExternalInput")
    wd_d = nc.dram_tensor("wd", [DT * 2, P, FT // 2, P], BF16, kind="ExternalInput")
    out_d = nc.dram_tensor("outT", [P, DT, T], F32, kind="ExternalOutput")

    AX = mybir.AxisListType.X
    AF = mybir.ActivationFunctionType
    ALU = mybir.AluOpType

    with tile.TileContext(nc) as tc:
        with (
            tc.tile_pool(name="const", bufs=1) as constp,
            tc.tile_pool(name="smv", bufs=1) as smv,
            tc.tile_pool(name="small", bufs=2) as small,
            tc.tile_pool(name="stream", bufs=6) as stream,
            tc.tile_pool(name="wdstream", bufs=4) as wdstream,
            tc.tile_pool(name="mid", bufs=1) as mid,
            tc.tile_pool(name="otp", bufs=2) as otp,
            tc.tile_pool(name="psum", bufs=4, space="PSUM") as psum,
            tc.tile_pool(name="psumA", bufs=2, space="PSUM") as psumA,
            tc.tile_pool(name="psumB", bufs=1, space="PSUM") as psumB,
        ):
            # ---- DMA triggers, ordered by need-time, spread over the three
            # trigger-capable queues (sync / scalar / gpsimd) ----
            # gpsimd: warm-up memset (no DMA dep) + scatter inputs
            warm_sb = constp.tile([P, T], BF16, tag="warm")
            nc.gpsimd.memset(warm_sb[:], 0.0)
            # sync: scatter indices first, then softmax table + x
            idxw = constp.tile([P, TT, 2 * K], BF16, tag="idxw")
            nc.sync.dma_start(idxw[:], idxw_d[:])
            blobf = constp.tile([P, BF_W], F32, tag="blobf")
            nc.sync.dma_start(blobf[:], blobf_d[:])
            xtb = constp.tile([P, DC, T], BF16, tag="xtb")
            for hh in range(2):
                h4 = ts(hh, DC // 2)
                nc.sync.dma_start(xtb[:, h4, :], xtb_d[:, h4, :])
            # scalar: the whole constant blob in one transfer; then exp of
            # the recipe table BEFORE the a1 triggers (a trailing dma_start
            # can stall the queue on the DMA ring, and exp must not wait)
            blobr = constp.tile([P, BR_W], BF16, tag="blobr")
            nc.scalar.dma_start(blobr[:], blobr_d[:])
            rec = blobf[:NN, BF_REC:BF_REC + NB]
            recsb = constp.tile([NN, NB], BF16, tag="recsb")
            ssum = small.tile([NN, 1], F32, tag="ssum")
            nc.scalar.activation(recsb[:], rec, AF.Exp, accum_out=ssum[:])
            a1 = constp.tile([P, DC, NB * R], BF16, tag="a1")
            for hh in range(2):
                h4 = ts(hh, DC // 2)
                nc.scalar.dma_start(a1[:, h4, :], a1_d[:, h4, :])
            # gpsimd SWDGE: slow (~10us) but a third parallel lane; a2 is not
            # needed until the delta matmuls (~22us)
            a2 = constp.tile([P, NRT, D], BF16, tag="a2")
            nc.gpsimd.dma_start(a2[:], a2_d[:])

            bu = blobf[:, BF_BU:BF_BU + FT]
            bd = blobf[:, BF_BD:BF_BD + DT]
            trep = blobr[:R, BR_TREP:BR_TREP + P]
            qred = blobr[:, BR_QRED:BR_QRED + R]
            identb = blobr[:, BR_ID:BR_ID + P]

            # ---- PE warm-up on the memset tile: starts immediately, no DMA
            # dependency, so the HAM clock gate reaches 8/8 right when the
            # first real matmul's data lands ----
            warm_ps = psumB.tile([P, T], F32, tag="htps", name="warm")
            for w in range(NWARM):
                nc.tensor.matmul(warm_ps[:], warm_sb[:, :P], warm_sb[:],
                                 start=(w == 0), stop=(w == NWARM - 1))

            # ---- routing scatter S[t, neuron] (weighted one-hot): one fused
            # 3-op Vector chain over all four token-tiles ----
            st_sb = constp.tile([NN, T], BF16, tag="st")
            iota4 = blobr[:, BR_I512:BR_I512 + NN * K].rearrange(
                "p (o n k) -> p o n k", o=1, k=K).to_broadcast((P, TT, NN, K))
            idx_b = idxw[:, :, 0:K].rearrange(
                "p t (o k) -> p t o k", o=1).to_broadcast((P, TT, NN, K))
            w_b = idxw[:, :, K:2 * K].rearrange(
                "p t (o k) -> p t o k", o=1).to_broadcast((P, TT, NN, K))
            sk = smv.tile([P, TT, NN, K], BF16, tag="sk")
            s_all = smv.tile([P, TT, NN], BF16, tag="s")
            nc.vector.tensor_tensor(sk[:], iota4, idx_b, ALU.is_equal)
            nc.vector.tensor_tensor(sk[:], sk[:], w_b, ALU.mult)
            # pairwise-tree reduction over k (faster than reduce_sum on DVE)
            nc.vector.tensor_tensor(sk[:, :, :, 0:4], sk[:, :, :, 0:4],
                                    sk[:, :, :, 4:8], ALU.add)
            nc.vector.tensor_tensor(sk[:, :, :, 0:2], sk[:, :, :, 0:2],
                                    sk[:, :, :, 2:4], ALU.add)
            nc.vector.tensor_tensor(
                s_all[:].rearrange("p t (n o) -> p t n o", o=1),
                sk[:, :, :, 0:1], sk[:, :, :, 1:2], ALU.add)
            rsum = small.tile([NN, 1], F32, tag="rsum")
            nc.vector.reciprocal(rsum[:], ssum[:])

            # anchor read for the warm-up matmuls (prevents dead-code elim)
            warm_anchor = small.tile([P, 1], F32, tag="warm_anchor")
            nc.vector.tensor_copy(warm_anchor[:], warm_ps[:, 0:1])

            # ---- recsT = recs^T, then M_i = (recs @ SEL_i) as [NN, P]
            # stationaries — all off the critical path (not scatter-gated) ----
            rT_ps = psumB.tile([NB, NN], BF16, tag="stp", name="rTps")
            nc.tensor.transpose(rT_ps[:], recsb[:], identb[:NN, :NN])
            recsT = constp.tile([NB, NN], BF16, tag="recsT")
            nc.scalar.activation(recsT[:], rT_ps[:], AF.Copy)

            yt_ps = [psum.tile([P, T], F32, tag="ps", name=f"yt{i}")
                     for i in range(NRT)]

            def yt_block(i):
                for dc in range(DC):
                    nc.tensor.matmul(yt_ps[i][:], a1[:, dc, ts(i, P)],
                                     xtb[:, dc, :],
                                     start=(dc == 0), stop=(dc == DC - 1))

            yt_block(0)

            m_sb = []
            for i in range(NRT):
                mp = psumA.tile([NN, P], F32, tag="rp", name=f"m{i}")
                nc.tensor.matmul(mp[:], recsT[:], blobr[:NB, ts(i, P)],
                                 start=True, stop=True)
                ms = constp.tile([NN, P], BF16, tag=f"m{i}", name=f"ms{i}")
                nc.scalar.activation(ms[:], mp[:], AF.Copy)
                m_sb.append(ms)

            # ---- routing transposes: all four into one PSUM tile, one drain ----
            stp_all = psumB.tile([NN, TT * P], BF16, tag="stp", name="stp")
            for tt in range(TT):
                nc.tensor.transpose(stp_all[:, ts(tt, P)], s_all[:, tt, :],
                                    identb)
            nc.scalar.activation(st_sb[:], stp_all[:], AF.Copy,
                                 scale=rsum[:, 0:1])

            # ---- RepR[i][(n,r), t] = tr[t, n(i,p)] = M_i^T @ S^T ----
            repr_sb = []
            rr_ps = []
            for i in range(NRT):
                rp = psumA.tile([P, T], F32, tag="rp", name=f"rp{i}")
                rr = constp.tile([P, T], BF16, tag=f"repr{i}", name=f"repr{i}")
                rr_ps.append(rp)
                repr_sb.append(rr)

            def repr_block(i):
                nc.tensor.matmul(rr_ps[i][:], m_sb[i][:], st_sb[:],
                                 start=True, stop=True)
                if i < 2:
                    nc.vector.tensor_copy(repr_sb[i][:], rr_ps[i][:])
                else:
                    nc.scalar.activation(repr_sb[i][:], rr_ps[i][:], AF.Copy)

            repr_block(0)
            repr_block(1)
            yt_block(1)
            repr_block(2)
            repr_block(3)
            yt_block(2)

            # ---- WYT = YT * RepR;  hT = sum_n WYT ----
            ht_ps = psumB.tile([R, T], F32, tag="htps")
            wyt = [mid.tile([P, T], BF16, tag=f"mid{i}", name=f"wyt{i}")
                   for i in range(NRT)]
            nc.vector.tensor_mul(out=wyt[0][:], in0=yt_ps[0][:], in1=repr_sb[0][:])
            nc.vector.tensor_mul(out=wyt[1][:], in0=yt_ps[1][:], in1=repr_sb[1][:])
            nc.tensor.matmul(ht_ps[:], qred, wyt[0][:], start=True, stop=False)
            nc.tensor.matmul(ht_ps[:], qred, wyt[1][:], start=False, stop=False)
            yt_block(3)
            nc.vector.tensor_mul(out=wyt[2][:], in0=yt_ps[2][:], in1=repr_sb[2][:])
            nc.vector.tensor_mul(out=wyt[3][:], in0=yt_ps[3][:], in1=repr_sb[3][:])
            nc.tensor.matmul(ht_ps[:], qred, wyt[2][:], start=False, stop=False)
            nc.tensor.matmul(ht_ps[:], qred, wyt[3][:], start=False, stop=True)
            ht_sb = constp.tile([R, T], BF16, tag="ht")
            nc.scalar.activation(ht_sb[:], ht_ps[:], AF.Copy)

            # ---- CT = RepH * RepR;  deltaT = (alpha*A2)^T @ CT (alpha is
            # folded into A2 on the host);  xf = x + deltaT ----
            rh_ps = psumA.tile([P, T], F32, tag="rp", name="rh")
            nc.tensor.matmul(rh_ps[:], trep, ht_sb[:], start=True, stop=True)
            ct = [mid.tile([P, T], BF16, tag=f"mid{i}", name=f"ct{i}")
                  for i in range(NRT)]
            for i in range(NRT):
                nc.vector.tensor_mul(out=ct[i][:], in0=rh_ps[:], in1=repr_sb[i][:])
            xf = constp.tile([P, DC, T], BF16, tag="a1", name="xf")
            for half in range(2):
                dts = list(range(half * 4, half * 4 + 4))
                dl_ps = {dt: psum.tile([P, T], F32, tag="ps", name=f"dl{dt}")
                         for dt in dts}
                # last two chunks: preload x via identity matmul; the drains
                # then run on Scalar so Vector only does the earlier adds
                pre = [dt for dt in dts if dt >= 6]
                for dt in pre:
                    nc.tensor.matmul(dl_ps[dt][:], identb, xtb[:, dt, :],
                                     start=True, stop=False)
                # i outer so the first delta matmuls only need ct[0]
                for i in range(NRT):
                    for dt in dts:
                        nc.tensor.matmul(dl_ps[dt][:], a2[:, i, ts(dt, P)],
                                         ct[i][:],
                                         start=(i == 0 and dt not in pre),
                                         stop=(i == NRT - 1))
                for dt in dts:
                    if dt in pre:
                        nc.scalar.activation(xf[:, dt, :], dl_ps[dt][:], AF.Copy)
                    else:
                        nc.vector.tensor_add(out=xf[:, dt, :], in0=dl_ps[dt][:],
                                             in1=xtb[:, dt, :])

            # prefetch the first two wd tiles now (on scalar, before the
            # gelus) so the down projection never waits on DMA at the
            # up->down boundary
            wd_tiles = {}
            for dt in range(2):
                for h in range(2):
                    wdt = wdstream.tile([P, FT // 2, P], BF16, tag="wd",
                                        name=f"wd{dt}_{h}")
                    nc.scalar.dma_start(wdt[:], wd_d[dt * 2 + h])
                    wd_tiles[(dt, h)] = wdt

            # ---- FFN up + exact gelu ----
            g = constp.tile([P, FT, T], BF16, tag="g")
            for ftp in range(FT // 2):
                wu = stream.tile([P, 2, DC, P], BF16, tag="wu", name=f"wu{ftp}")
                nc.sync.dma_start(wu[:], wu_d[ftp])
                for j in range(2):
                    ft = 2 * ftp + j
                    u_ps = psum.tile([P, T], F32, tag="ps", name=f"u{ft}")
                    for dc in range(DC):
                        nc.tensor.matmul(u_ps[:], wu[:, j, dc, :], xf[:, dc, :],
                                         start=(dc == 0), stop=(dc == DC - 1))
                    nc.scalar.activation(g[:, ft, :], u_ps[:], AF.Gelu,
                                         bias=bu[:, ft:ft + 1], scale=1.0)

            # ---- FFN down + bias; bias-add split Vector/Scalar and the
            # output DMA split across two queues to shorten the tail ----
            TH = T // 2
            for dt in range(DT):
                o_ps = psum.tile([P, T], F32, tag="ps", name=f"o{dt}")
                for h in range(2):
                    wd = wd_tiles.pop((dt, h))
                    for fc in range(FT // 2):
                        fcg = h * (FT // 2) + fc
                        nc.tensor.matmul(o_ps[:], wd[:, fc, :], g[:, fcg, :],
                                         start=(fcg == 0), stop=(fcg == FT - 1))
                    if dt + 2 < DT:
                        wdt = wdstream.tile([P, FT // 2, P], BF16, tag="wd",
                                            name=f"wd{dt + 2}_{h}")
                        nc.scalar.dma_start(wdt[:], wd_d[(dt + 2) * 2 + h])
                        wd_tiles[(dt + 2, h)] = wdt
                otv = otp.tile([P, TH], F32, tag="otv", name=f"otv{dt}")
                nc.vector.tensor_scalar_add(otv[:], o_ps[:, 0:TH],
                                            bd[:, dt:dt + 1])
                nc.sync.dma_start(out_d[:, dt, 0:TH], otv[:])
                otg = otp.tile([P, TH], F32, tag="otg", name=f"otg{dt}")
                nc.scalar.activation(otg[:], o_ps[:, TH:T], AF.Identity,
                                     bias=bd[:, dt:dt + 1], scale=1.0)
                nc.scalar.dma_start(out_d[:, dt, TH:T], otg[:])

    nc.finalize()
    return nc


def _get_nc():
    if _BUILT[0] is None:
        _BUILT[0] = _build_nc()
    return _BUILT[0]


def kernel(x, neuron_idx, neuron_weights, neuron_recipe, basis_A,
           w_up_w, w_up_b, w_down_w, w_down_b, alpha):
    import ml_dtypes
    nc = _get_nc()

    x = np.asarray(x, dtype=np.float32).reshape(NCORES * T, D)
    idxf = np.asarray(neuron_idx).astype(np.float32).reshape(NCORES * T, K)
    wgt = np.asarray(neuron_weights, dtype=np.float32).reshape(NCORES * T, K)
    rec = np.asarray(neuron_recipe, dtype=np.float32)
    bA = np.asarray(basis_A, dtype=np.float32)
    wu = np.asarray(w_up_w, dtype=np.float32)
    bu_in = np.asarray(w_up_b, dtype=np.float32)
    wd = np.asarray(w_down_w, dtype=np.float32)
    bd_in = np.asarray(w_down_b, dtype=np.float32)
    alpha_f = float(np.asarray(alpha, dtype=np.float32))

    # replicated operands, packed into the on-device layouts
    a1 = np.ascontiguousarray(
        bA.transpose(1, 0, 2).reshape(D, NB * R)
        .reshape(DC, P, NB * R).transpose(1, 0, 2)).astype(ml_dtypes.bfloat16)
    # alpha folded into A2 so no on-device scaling is needed
    a2 = np.ascontiguousarray(
        (bA.transpose(0, 2, 1).reshape(NB * R, D) * alpha_f)
        .reshape(NRT, P, D).transpose(1, 0, 2)).astype(ml_dtypes.bfloat16)
    wu_p = np.ascontiguousarray(
        wu.reshape(DC, P, FT // 2, 2, P).transpose(2, 1, 3, 0, 4)
    ).astype(ml_dtypes.bfloat16)
    wd_p = np.ascontiguousarray(
        wd.reshape(2, FT // 2, P, DT, P).transpose(3, 0, 2, 1, 4)
        .reshape(DT * 2, P, FT // 2, P)).astype(ml_dtypes.bfloat16)

    blobf = np.zeros((P, BF_W), dtype=np.float32)
    blobf[:, BF_ID:BF_ID + P] = np.eye(P, dtype=np.float32)
    blobf[:, BF_BU:BF_BU + FT] = bu_in.reshape(FT, P).T
    blobf[:, BF_BD:BF_BD + DT] = bd_in.reshape(DT, P).T
    blobf[:, BF_AL] = alpha_f
    blobf[:NN, BF_REC:BF_REC + NB] = rec

    blobr = np.zeros((P, BR_W), dtype=np.float32)
    # SEL[n, i*128+m] = 1 iff n in [4i, 4i+4) and m // 32 == n - 4i
    for n in range(NB):
        i, nloc = divmod(n, NRT)
        blobr[n, BR_SEL + i * P + nloc * R: BR_SEL + i * P + (nloc + 1) * R] = 1.0
    blobr[:R, BR_TREP:BR_TREP + P] = (
        np.arange(P)[None, :] % R == np.arange(R)[:, None])
    blobr[:, BR_QRED:BR_QRED + R] = (
        np.arange(P)[:, None] % R == np.arange(R)[None, :])
    blobr[:, BR_IOTA:BR_IOTA + NN] = np.arange(NN, dtype=np.float32)[None, :]
    blobr[:, BR_ID:BR_ID + P] = np.eye(P, dtype=np.float32)
    blobr[:, BR_I512:BR_I512 + NN * K] = np.repeat(
        np.arange(NN, dtype=np.float32), K)[None, :]
    blobr = blobr.astype(ml_dtypes.bfloat16)

    shared = {
        "blobf": blobf, "blobr": blobr, "a1": a1, "a2": a2,
        "wu": wu_p, "wd": wd_p,
    }
    in_maps = []
    idxw = np.concatenate([idxf, wgt], axis=1).astype(
        ml_dtypes.bfloat16)  # [N*T, 16]
    for c in range(NCORES):
        xc = x[c * T:(c + 1) * T]  # [T, D]
        xtc = np.ascontiguousarray(xc.T.reshape(DC, P, T).transpose(1, 0, 2))
        xtbc = xtc.astype(ml_dtypes.bfloat16)
        iwc = np.ascontiguousarray(
            idxw[c * T:(c + 1) * T].reshape(TT, P, 2 * K).transpose(1, 0, 2))
        in_maps.append({"xtb": xtbc, "idxw": iwc, **shared})

    res = run_bass_kernel_spmd(nc, in_maps, core_ids=list(range(NCORES)))

    out = np.empty((NCORES * T, D), dtype=np.float32)
    for c in range(NCORES):
        ot = res.results[c]["outT"]  # [P, DT, T]
        out[c * T:(c + 1) * T] = ot.transpose(1, 0, 2).reshape(D, T).T
    return out.reshape(2, 2048, D)



# revision 6
# speedup vs baseline: 1.0101x; 1.0101x over previous
"""BasisResidualFFN Trainium2 kernel.

Math (per token t):
  recipe_soft = softmax(neuron_recipe, axis=-1)                 [64, 16]
  tr[t, :]    = sum_k w[t,k] * recipe_soft[idx[t,k], :]         [16]
  Y[t, (n,r)] = sum_d x[t,d] * basis_A[n,d,r]
  h[t, r]     = sum_n tr[t,n] * Y[t,(n,r)]
  delta[t, d] = sum_{n,r} basis_A[n,d,r] * tr[t,n] * h[t,r]
  out         = gelu((x + alpha*delta) @ w_up + b_up) @ w_down + b_down

Distribution: pure data parallel. B*S = 4096 tokens sharded 512/core
across 8 NeuronCores; all weights replicated. Everything on device is
computed feature-major (features on partitions, tokens on the free
axis) so no on-device activation transposes are needed; x arrives
pre-transposed from the host and the output is un-transposed there.

Precision: bf16 everywhere except the delta projection, which runs as
fp8 e4m3 DoubleRow matmuls (2x PE throughput): delta = (32*alpha*A2)^T
@ ct with ct in fp8 and the 1/32 compensation riding the PSUM drain
(x is preloaded into PSUM via a 32*identity matmul, so the drain is a
single scaled copy). delta errors enter only through alpha*delta with
alpha ~ 0.1, costing ~1e-4 extra rel err (measured 4e-3 total).

Schedule: the PE clock ramps over ~3us of continuous busy and
re-throttles after idle, so the PE must never stall. x and a1 stream
in dc-chunks and the YT matmuls run dc-outer, consuming each chunk as
it lands right behind the DMA; the routing matmuls (recsT, M_i, S^T
transposes, RepR) are interleaved between YT chunks so the tensor
queue stays dense while Vector runs the routing scatter. The FFN
weights stream behind, double-buffered, and the output leaves as bf16.
"""

import numpy as np

import concourse.bass as bass
import concourse.mybir as mybir
import concourse.tile as tile
from concourse import bacc
from concourse.bass import ts
from concourse.bass_utils import run_bass_kernel_spmd

P = 128
NCORES = 8
T = 512            # tokens per core
D = 1024
DFF = 4096
NB = 16            # n_basis
R = 32             # rank
NN = 64            # n_neurons
K = 8              # top-k
DC = D // P        # 8 contraction chunks over d
FT = DFF // P      # 32 ff tiles
DT = D // P        # 8 output d tiles
NRT = (NB * R) // P  # 4 (n,r) tiles
TT = T // P        # 4 token tiles per core

# const blob column layouts (bf16 blob / f32 blob)
BR_TREP, BR_QRED, BR_ID, BR_I512, BR_ID32, BR_W = 0, 128, 160, 288, 800, 928
BF_BU, BF_BD, BF_REC, BF_W = 0, 32, 40, 56

F32 = mybir.dt.float32
BF16 = mybir.dt.bfloat16
F8 = mybir.dt.float8e4

DR = mybir.MatmulPerfMode.DoubleRow

NWARM = 3
A2S = 32.0         # fp8 scale on alpha*A2; compensated in the xf drain

_BUILT = [None]


def _build_nc():
    nc = bacc.Bacc(None, target_bir_lowering=False)

    xtb_d = nc.dram_tensor("xtb", [P, DC, T], BF16, kind="ExternalInput")
    idxw_d = nc.dram_tensor("idxw", [P, TT, 2 * K], BF16, kind="ExternalInput")
    blobr_d = nc.dram_tensor("blobr", [P, BR_W], BF16, kind="ExternalInput")
    blobf_d = nc.dram_tensor("blobf", [P, BF_W], F32, kind="ExternalInput")
    sel_d = nc.dram_tensor("sel", [NB, NRT, P], BF16, kind="ExternalInput")
    a1_d = nc.dram_tensor("a1", [P, DC, NB * R], BF16, kind="ExternalInput")
    a2_d = nc.dram_tensor("a2", [P, 2, 2, DT, P], F8, kind="ExternalInput")
    wu_d = nc.dram_tensor("wu", [FT // 2, P, 2, DC, P], BF16, kind="ExternalInput")
    wd_d = nc.dram_tensor("wd", [DT * 2, P, FT // 2, P], BF16, kind="ExternalInput")
    out_d = nc.dram_tensor("outT", [P, DT, T], BF16, kind="ExternalOutput")

    AF = mybir.ActivationFunctionType
    ALU = mybir.AluOpType

    with tile.TileContext(nc) as tc:
        with (
            tc.tile_pool(name="const", bufs=1) as constp,
            tc.tile_pool(name="smv", bufs=1) as smv,
            tc.tile_pool(name="small", bufs=2) as small,
            tc.tile_pool(name="stream", bufs=6) as stream,
            tc.tile_pool(name="wdstream", bufs=4) as wdstream,
            tc.tile_pool(name="otp", bufs=2) as otp,
            tc.tile_pool(name="psum", bufs=5, space="PSUM") as psum,
            tc.tile_pool(name="psumA", bufs=2, space="PSUM") as psumA,
            tc.tile_pool(name="psumB", bufs=1, space="PSUM") as psumB,
        ):
            # ---- DMA triggers, ordered by need-time. sync ring: routing
            # consts then x; scalar ring: a1 chunks then a2; gpsimd only
            # memsets (SWDGE is too slow for anything on the path) ----
            warm_sb = constp.tile([P, T], BF16, tag="warm")
            nc.gpsimd.memset(warm_sb[:], 0.0)
            idxw = constp.tile([P, TT, 2 * K], BF16, tag="idxw")
            nc.sync.dma_start(idxw[:], idxw_d[:])
            blobr = constp.tile([P, BR_W], BF16, tag="blobr")
            nc.sync.dma_start(blobr[:], blobr_d[:])
            sel = constp.tile([NB, NRT, P], BF16, tag="sel")
            nc.sync.dma_start(sel[:], sel_d[:])
            xtb = constp.tile([P, DC, T], BF16, tag="xtb")
            for hh in range(4):
                h2 = ts(hh, DC // 4)
                nc.sync.dma_start(xtb[:, h2, :], xtb_d[:, h2, :])

            blobf = constp.tile([P, BF_W], F32, tag="blobf")
            nc.scalar.dma_start(blobf[:], blobf_d[:])
            a1 = constp.tile([P, DC, NB * R], BF16, tag="a1")
            for hh in range(2):
                h2 = ts(hh, DC // 4)
                nc.scalar.dma_start(a1[:, h2, :], a1_d[:, h2, :])
            # exp of the recipe table early (needs only blobf)
            rec = blobf[:NN, BF_REC:BF_REC + NB]
            recsb = constp.tile([NN, NB], BF16, tag="recsb")
            ssum = small.tile([NN, 1], F32, tag="ssum")
            nc.scalar.activation(recsb[:], rec, AF.Exp, accum_out=ssum[:])
            for hh in range(2, 4):
                h2 = ts(hh, DC // 4)
                nc.scalar.dma_start(a1[:, h2, :], a1_d[:, h2, :])
            a2 = constp.tile([P, 2, 2, DT, P], F8, tag="a2")
            nc.scalar.dma_start(a2[:], a2_d[:])
            # anchor read for the warm-up matmuls (prevents dead-code elim);
            # on Scalar so it cannot delay the Vector scatter or the PE
            warm_anchor = small.tile([P, 1], F32, tag="warm_anchor")

            bu = blobf[:, BF_BU:BF_BU + FT]
            bd = blobf[:, BF_BD:BF_BD + DT]
            trep = blobr[:R, BR_TREP:BR_TREP + P]
            qred = blobr[:, BR_QRED:BR_QRED + R]
            identb = blobr[:, BR_ID:BR_ID + P]
            ident32 = blobr[:, BR_ID32:BR_ID32 + P]

            # ---- PE warm-up on the memset tile: starts at user-code time
            # zero with no DMA dependency so the clock ramp begins before the
            # first x/a1 chunk lands ----
            warm_ps = psumB.tile([P, T], F32, tag="b", name="warm")
            for w in range(NWARM):
                nc.tensor.matmul(warm_ps[:], warm_sb[:, :P], warm_sb[:],
                                 start=(w == 0), stop=(w == NWARM - 1))
            nc.scalar.activation(warm_anchor[:], warm_ps[:, 0:1], AF.Copy)

            # ---- routing scatter S[t, neuron] (weighted one-hot): one fused
            # 3-op Vector chain over all four token-tiles ----
            st_sb = constp.tile([NN, T], BF16, tag="st")
            iota4 = blobr[:, BR_I512:BR_I512 + NN * K].rearrange(
                "p (o n k) -> p o n k", o=1, k=K).to_broadcast((P, TT, NN, K))
            idx_b = idxw[:, :, 0:K].rearrange(
                "p t (o k) -> p t o k", o=1).to_broadcast((P, TT, NN, K))
            w_b = idxw[:, :, K:2 * K].rearrange(
                "p t (o k) -> p t o k", o=1).to_broadcast((P, TT, NN, K))
            sk = smv.tile([P, TT, NN, K], BF16, tag="sk")
            s_all = smv.tile([P, TT, NN], BF16, tag="s")
            nc.vector.tensor_tensor(sk[:], iota4, idx_b, ALU.is_equal)
            nc.vector.tensor_tensor(sk[:], sk[:], w_b, ALU.mult)
            # pairwise-tree reduction over k (faster than reduce_sum on DVE)
            nc.vector.tensor_tensor(sk[:, :, :, 0:4], sk[:, :, :, 0:4],
                                    sk[:, :, :, 4:8], ALU.add)
            nc.vector.tensor_tensor(sk[:, :, :, 0:2], sk[:, :, :, 0:2],
                                    sk[:, :, :, 2:4], ALU.add)
            nc.vector.tensor_tensor(
                s_all[:].rearrange("p t (n o) -> p t n o", o=1),
                sk[:, :, :, 0:1], sk[:, :, :, 1:2], ALU.add)
            rsum = small.tile([NN, 1], F32, tag="rsum")
            nc.vector.reciprocal(rsum[:], ssum[:])

            yt_ps = [psum.tile([P, T], F32, tag="ps", name=f"yt{i}")
                     for i in range(NRT)]

            def yt_chunk(dc):
                for i in range(NRT):
                    nc.tensor.matmul(yt_ps[i][:], a1[:, dc, ts(i, P)],
                                     xtb[:, dc, :],
                                     start=(dc == 0), stop=(dc == DC - 1))

            # ---- YT chunks follow the x/a1 DMA; routing matmuls fill the
            # gaps between chunks ----
            yt_chunk(0)

            # recsT = recs^T, then M_i = (recs @ SEL_i)^T as [NN, P]
            rT_ps = psumB.tile([NB, NN], BF16, tag="b", name="rTps")
            nc.tensor.transpose(rT_ps[:], recsb[:], identb[:NN, :NN])
            recsT = constp.tile([NB, NN], BF16, tag="recsT")
            nc.scalar.activation(recsT[:], rT_ps[:], AF.Copy)

            yt_chunk(1)

            m_sb = []
            for i in range(NRT):
                mp = psumA.tile([NN, P], F32, tag="rp", name=f"m{i}")
                nc.tensor.matmul(mp[:], recsT[:], sel[:, i, :],
                                 start=True, stop=True)
                ms = constp.tile([NN, P], BF16, tag=f"m{i}", name=f"ms{i}")
                nc.scalar.activation(ms[:], mp[:], AF.Copy)
                m_sb.append(ms)

            yt_chunk(2)
            yt_chunk(3)
            yt_chunk(4)

            # routing transposes: all four into one PSUM tile, one drain
            stp_all = psumB.tile([NN, TT * P], BF16, tag="b", name="stp")
            for tt in range(TT):
                nc.tensor.transpose(stp_all[:, ts(tt, P)], s_all[:, tt, :],
                                    identb)
            nc.scalar.activation(st_sb[:], stp_all[:], AF.Copy,
                                 scale=rsum[:, 0:1])

            yt_chunk(5)

            # RepR[i][(n,r), t] = tr[t, n(i,p)] = M_i^T @ S^T, into one tile
            reprall = constp.tile([P, NRT, T], BF16, tag="reprall")
            rr_ps = [psumA.tile([P, T], F32, tag="rp", name=f"rp{i}")
                     for i in range(NRT)]

            def repr_block(i):
                nc.tensor.matmul(rr_ps[i][:], m_sb[i][:], st_sb[:],
                                 start=True, stop=True)
                if i % 2 == 0:
                    nc.vector.tensor_copy(reprall[:, i, :], rr_ps[i][:])
                else:
                    nc.scalar.activation(reprall[:, i, :], rr_ps[i][:], AF.Copy)

            yt_chunk(6)
            repr_block(0)
            repr_block(1)
            yt_chunk(7)
            repr_block(2)
            repr_block(3)

            # ---- WYT = YT * RepR;  hT = sum_n WYT ----
            ht_ps = psumB.tile([R, T], F32, tag="b", name="htps")
            wyt = constp.tile([P, NRT, T], BF16, tag="wyt")
            for i in range(NRT):
                nc.vector.tensor_mul(out=wyt[:, i, :], in0=yt_ps[i][:],
                                     in1=reprall[:, i, :])
                nc.tensor.matmul(ht_ps[:], qred, wyt[:, i, :],
                                 start=(i == 0), stop=(i == NRT - 1))
            ht_sb = constp.tile([R, T], BF16, tag="ht")
            nc.scalar.activation(ht_sb[:], ht_ps[:], AF.Copy)

            # ---- preload 32*x into PSUM (so the xf drain is one scaled
            # copy); these identity matmuls fill PE gaps while Vector builds
            # ct ----
            xf = constp.tile([P, DC, T], BF16, tag="a1", name="xf")
            dl_ps = {}
            for dt in range(2):
                dl_ps[dt] = psum.tile([P, T], F32, tag="ps", name=f"dl{dt}")
                nc.tensor.matmul(dl_ps[dt][:], ident32, xtb[:, dt, :],
                                 start=True, stop=False)

            # CT = RepH * RepR in fp8 (two halves so the first DoubleRow
            # pair can start early); RepH = trep @ hT
            rh_ps = psumA.tile([P, T], F32, tag="rp", name="rh")
            nc.tensor.matmul(rh_ps[:], trep, ht_sb[:], start=True, stop=True)
            for dt in range(2, 5):
                dl_ps[dt] = psum.tile([P, T], F32, tag="ps", name=f"dl{dt}")
                nc.tensor.matmul(dl_ps[dt][:], ident32, xtb[:, dt, :],
                                 start=True, stop=False)
            ct = constp.tile([P, NRT, T], F8, tag="ct")
            for i in range(NRT):
                nc.vector.tensor_tensor(ct[:, i, :], rh_ps[:],
                                        reprall[:, i, :], ALU.mult)

            # ---- deltaT: fp8 DoubleRow, dt-outer so drains chase;  xf =
            # (32*x + 32*alpha*delta) / 32 on alternating Scalar/Vector ----
            def drain_xf(dt):
                if dt % 2 == 0:
                    nc.scalar.activation(xf[:, dt, :], dl_ps[dt][:], AF.Copy,
                                         scale=1.0 / A2S)
                else:
                    nc.vector.tensor_scalar_mul(xf[:, dt, :], dl_ps[dt][:],
                                                1.0 / A2S)

            def delta_block(dt):
                for pi in range(2):
                    nc.tensor.matmul(dl_ps[dt][:], a2[:, pi, :, dt, :],
                                     ct[:, 2 * pi:2 * pi + 2, :],
                                     start=False, stop=(pi == 1),
                                     perf_mode=DR)
                drain_xf(dt)

            for dt in range(3):
                delta_block(dt)
            for dt in range(5, DT):
                dl_ps[dt] = psum.tile([P, T], F32, tag="ps", name=f"dl{dt}")
                nc.tensor.matmul(dl_ps[dt][:], ident32, xtb[:, dt, :],
                                 start=True, stop=False)
                delta_block(dt - 2)
            delta_block(6)
            delta_block(7)

            # prefetch the first two wd tiles now (on scalar, before the
            # gelus) so the down projection never waits on DMA at the
            # up->down boundary
            wd_tiles = {}
            for dt in range(2):
                for h in range(2):
                    wdt = wdstream.tile([P, FT // 2, P], BF16, tag="wd",
                                        name=f"wd{dt}_{h}")
                    nc.scalar.dma_start(wdt[:], wd_d[dt * 2 + h])
                    wd_tiles[(dt, h)] = wdt

            # ---- FFN up + exact gelu ----
            g = constp.tile([P, FT, T], BF16, tag="g")
            for ftp in range(FT // 2):
                wu = stream.tile([P, 2, DC, P], BF16, tag="wu", name=f"wu{ftp}")
                nc.sync.dma_start(wu[:], wu_d[ftp])
                for j in range(2):
                    ft = 2 * ftp + j
                    u_ps = psum.tile([P, T], F32, tag="ps", name=f"u{ft}")
                    for dc in range(DC):
                        nc.tensor.matmul(u_ps[:], wu[:, j, dc, :], xf[:, dc, :],
                                         start=(dc == 0), stop=(dc == DC - 1))
                    nc.scalar.activation(g[:, ft, :], u_ps[:], AF.Gelu,
                                         bias=bu[:, ft:ft + 1], scale=1.0)

            # ---- FFN down + bias; bias-add split Vector/Scalar and the
            # output DMA split across two queues to shorten the tail ----
            TH = T // 2
            for dt in range(DT):
                o_ps = psum.tile([P, T], F32, tag="ps", name=f"o{dt}")
                for h in range(2):
                    wd = wd_tiles.pop((dt, h))
                    for fc in range(FT // 2):
                        fcg = h * (FT // 2) + fc
                        nc.tensor.matmul(o_ps[:], wd[:, fc, :], g[:, fcg, :],
                                         start=(fcg == 0), stop=(fcg == FT - 1))
                    if dt + 2 < DT:
                        wdt = wdstream.tile([P, FT // 2, P], BF16, tag="wd",
                                            name=f"wd{dt + 2}_{h}")
                        nc.scalar.dma_start(wdt[:], wd_d[(dt + 2) * 2 + h])
                        wd_tiles[(dt + 2, h)] = wdt
                otv = otp.tile([P, TH], BF16, tag="otv", name=f"otv{dt}")
                nc.vector.tensor_scalar_add(otv[:], o_ps[:, 0:TH],
                                            bd[:, dt:dt + 1])
                nc.sync.dma_start(out_d[:, dt, 0:TH], otv[:])
                otg = otp.tile([P, TH], BF16, tag="otg", name=f"otg{dt}")
                nc.scalar.activation(otg[:], o_ps[:, TH:T], AF.Identity,
                                     bias=bd[:, dt:dt + 1], scale=1.0)
                nc.scalar.dma_start(out_d[:, dt, TH:T], otg[:])

    nc.finalize()
    return nc


def _get_nc():
    if _BUILT[0] is None:
        _BUILT[0] = _build_nc()
    return _BUILT[0]


def kernel(x, neuron_idx, neuron_weights, neuron_recipe, basis_A,
           w_up_w, w_up_b, w_down_w, w_down_b, alpha):
    import ml_dtypes
    nc = _get_nc()

    x = np.asarray(x, dtype=np.float32).reshape(NCORES * T, D)
    idxf = np.asarray(neuron_idx).astype(np.float32).reshape(NCORES * T, K)
    wgt = np.asarray(neuron_weights, dtype=np.float32).reshape(NCORES * T, K)
    rec = np.asarray(neuron_recipe, dtype=np.float32)
    bA = np.asarray(basis_A, dtype=np.float32)
    wu = np.asarray(w_up_w, dtype=np.float32)
    bu_in = np.asarray(w_up_b, dtype=np.float32)
    wd = np.asarray(w_down_w, dtype=np.float32)
    bd_in = np.asarray(w_down_b, dtype=np.float32)
    alpha_f = float(np.asarray(alpha, dtype=np.float32))

    # replicated operands, packed into the on-device layouts
    a1 = np.ascontiguousarray(
        bA.transpose(1, 0, 2).reshape(D, NB * R)
        .reshape(DC, P, NB * R).transpose(1, 0, 2)).astype(ml_dtypes.bfloat16)
    # delta projection in fp8: 32*alpha*A2, [(pair, j, p), (dt, c)] packed
    a2m = np.clip(bA.transpose(0, 2, 1).reshape(NB * R, D) * (alpha_f * A2S),
                  -240.0, 240.0)
    a2 = np.ascontiguousarray(
        a2m.reshape(2, 2, P, DT, P).transpose(2, 0, 1, 3, 4)
    ).astype(ml_dtypes.float8_e4m3)
    wu_p = np.ascontiguousarray(
        wu.reshape(DC, P, FT // 2, 2, P).transpose(2, 1, 3, 0, 4)
    ).astype(ml_dtypes.bfloat16)
    wd_p = np.ascontiguousarray(
        wd.reshape(2, FT // 2, P, DT, P).transpose(3, 0, 2, 1, 4)
        .reshape(DT * 2, P, FT // 2, P)).astype(ml_dtypes.bfloat16)

    blobf = np.zeros((P, BF_W), dtype=np.float32)
    blobf[:, BF_BU:BF_BU + FT] = bu_in.reshape(FT, P).T
    blobf[:, BF_BD:BF_BD + DT] = bd_in.reshape(DT, P).T
    blobf[:NN, BF_REC:BF_REC + NB] = rec

    blobr = np.zeros((P, BR_W), dtype=np.float32)
    blobr[:R, BR_TREP:BR_TREP + P] = (
        np.arange(P)[None, :] % R == np.arange(R)[:, None])
    blobr[:, BR_QRED:BR_QRED + R] = (
        np.arange(P)[:, None] % R == np.arange(R)[None, :])
    blobr[:, BR_ID:BR_ID + P] = np.eye(P, dtype=np.float32)
    blobr[:, BR_I512:BR_I512 + NN * K] = np.repeat(
        np.arange(NN, dtype=np.float32), K)[None, :]
    blobr[:, BR_ID32:BR_ID32 + P] = np.eye(P, dtype=np.float32) * A2S
    blobr = blobr.astype(ml_dtypes.bfloat16)

    # SEL[n, i, m] = 1 iff n in [4i, 4i+4) and m // 32 == n - 4i
    sel = np.zeros((NB, NRT, P), dtype=np.float32)
    for n in range(NB):
        i, nloc = divmod(n, NRT)
        sel[n, i, nloc * R:(nloc + 1) * R] = 1.0
    sel = sel.astype(ml_dtypes.bfloat16)

    shared = {
        "blobf": blobf, "blobr": blobr, "sel": sel, "a1": a1, "a2": a2,
        "wu": wu_p, "wd": wd_p,
    }
    in_maps = []
    idxw = np.concatenate([idxf, wgt], axis=1).astype(
        ml_dtypes.bfloat16)  # [N*T, 16]
    for c in range(NCORES):
        xc = x[c * T:(c + 1) * T]  # [T, D]
        xtc = np.ascontiguousarray(xc.T.reshape(DC, P, T).transpose(1, 0, 2))
        xtbc = xtc.astype(ml_dtypes.bfloat16)
        iwc = np.ascontiguousarray(
            idxw[c * T:(c + 1) * T].reshape(TT, P, 2 * K).transpose(1, 0, 2))
        in_maps.append({"xtb": xtbc, "idxw": iwc, **shared})

    res = run_bass_kernel_spmd(nc, in_maps, core_ids=list(range(NCORES)))

    out = np.empty((NCORES * T, D), dtype=np.float32)
    for c in range(NCORES):
        ot = res.results[c]["outT"].astype(np.float32)  # [P, DT, T]
        out[c * T:(c + 1) * T] = ot.transpose(1, 0, 2).reshape(D, T).T
    return out.reshape(2, 2048, D)


# revision 17
# speedup vs baseline: 1.0442x; 1.0338x over previous
"""BasisResidualFFN Trainium2 kernel.

Math (per token t):
  recipe_soft = softmax(neuron_recipe, axis=-1)                 [64, 16]
  tr[t, :]    = sum_k w[t,k] * recipe_soft[idx[t,k], :]         [16]
  Y[t, (n,r)] = sum_d x[t,d] * basis_A[n,d,r]
  h[t, r]     = sum_n tr[t,n] * Y[t,(n,r)]
  delta[t, d] = sum_{n,r} basis_A[n,d,r] * tr[t,n] * h[t,r]
  out         = gelu((x + alpha*delta) @ w_up + b_up) @ w_down + b_down

Distribution: pure data parallel. B*S = 4096 tokens sharded 512/core
across 8 NeuronCores; all weights replicated. Everything on device is
computed feature-major (features on partitions, tokens on the free
axis) so no on-device activation transposes are needed; x arrives
pre-transposed from the host and the output is un-transposed there.

Precision: bf16 everywhere except the delta projection, which runs as
fp8 e4m3 DoubleRow matmuls (2x PE throughput): delta = (32*alpha*A2)^T
@ ct with ct in fp8 and the 1/32 compensation riding the PSUM drain
(x is preloaded into PSUM via a 32*identity matmul, so the drain is a
single scaled copy). delta errors enter only through alpha*delta with
alpha ~ 0.1, costing ~1e-4 extra rel err (measured 4e-3 total).

Schedule: the PE clock ramps over ~3us of continuous busy and
re-throttles after idle, so the PE must never stall. x and a1 stream
in dc-chunks and the YT matmuls run dc-outer, consuming each chunk as
it lands right behind the DMA; the routing matmuls (recsT, M_i, S^T
transposes, RepR) are interleaved between YT chunks so the tensor
queue stays dense while Vector runs the routing scatter. The FFN
weights stream behind, double-buffered, and the output leaves as bf16.
"""

import numpy as np

import concourse.bass as bass
import concourse.mybir as mybir
import concourse.tile as tile
from concourse import bacc
from concourse.bass import ts
from concourse.bass_utils import run_bass_kernel_spmd

P = 128
NCORES = 8
T = 512            # tokens per core
D = 1024
DFF = 4096
NB = 16            # n_basis
R = 32             # rank
NN = 64            # n_neurons
K = 8              # top-k
DC = D // P        # 8 contraction chunks over d
FT = DFF // P      # 32 ff tiles
DT = D // P        # 8 output d tiles
NRT = (NB * R) // P  # 4 (n,r) tiles
TT = T // P        # 4 token tiles per core

# const blob column layouts (bf16 blob / f32 blob)
BR_TREP, BR_QRED, BR_ID, BR_ID32, BR_W = 0, 128, 160, 288, 416
BF_BU, BF_BD, BF_REC, BF_W = 0, 32, 40, 56

F32 = mybir.dt.float32
BF16 = mybir.dt.bfloat16
F8 = mybir.dt.float8e4

DR = mybir.MatmulPerfMode.DoubleRow

NWARM = 8          # keeps the PE busy (and its clock ramped) until x lands
A2S = 32.0         # fp8 scale on alpha*A2; compensated in the xf drain
A1S = 16.0         # fp8 scale on A1; compensated in qred

_BUILT = [None]


def _build_nc():
    nc = bacc.Bacc(None, target_bir_lowering=False)

    xtb_d = nc.dram_tensor("xtb", [P, DC, T], BF16, kind="ExternalInput")
    idxw_d = nc.dram_tensor("idxw", [P, TT, 2 * K], BF16, kind="ExternalInput")
    blobr_d = nc.dram_tensor("blobr", [P, BR_W], BF16, kind="ExternalInput")
    blobf_d = nc.dram_tensor("blobf", [P, BF_W], F32, kind="ExternalInput")
    sel_d = nc.dram_tensor("sel", [NB, NRT, P], BF16, kind="ExternalInput")
    a1_d = nc.dram_tensor("a1", [P, DC, NB * R], F8, kind="ExternalInput")
    a2_d = nc.dram_tensor("a2", [P, 2, 2, DT, P], F8, kind="ExternalInput")
    wu_d = nc.dram_tensor("wu", [FT // 2, P, 2, DC, P], BF16, kind="ExternalInput")
    wd_d = nc.dram_tensor("wd", [DT * 2, P, FT // 2, P], BF16, kind="ExternalInput")
    out_d = nc.dram_tensor("outT", [P, DT, T], BF16, kind="ExternalOutput")

    AF = mybir.ActivationFunctionType
    ALU = mybir.AluOpType

    with tile.TileContext(nc) as tc:
        with (
            tc.tile_pool(name="const", bufs=1) as constp,
            tc.tile_pool(name="smv", bufs=1) as smv,
            tc.tile_pool(name="small", bufs=2) as small,
            tc.tile_pool(name="stream", bufs=6) as stream,
            tc.tile_pool(name="wdstream", bufs=4) as wdstream,
            tc.tile_pool(name="otp", bufs=2) as otp,
            tc.tile_pool(name="psum", bufs=5, space="PSUM") as psum,
            tc.tile_pool(name="psumA", bufs=2, space="PSUM") as psumA,
            tc.tile_pool(name="psumB", bufs=1, space="PSUM") as psumB,
        ):
            # ---- DMA triggers, ordered by need-time. sync ring: routing
            # consts then x; scalar ring: a1 chunks then a2; gpsimd only
            # memsets (SWDGE is too slow for anything on the path) ----
            warm_sb = constp.tile([P, T], BF16, tag="warm")
            nc.gpsimd.memset(warm_sb[:], 0.0)
            # iota table for the scatter, generated on the (otherwise idle)
            # GpSimd lane instead of DMA'd: repeat(arange(64), 8)
            i512 = constp.tile([P, NN * K], BF16, tag="i512")
            nc.gpsimd.iota(i512[:], pattern=[[1, NN], [0, K]], base=0,
                           channel_multiplier=0,
                           allow_small_or_imprecise_dtypes=True)
            idxw = constp.tile([P, TT, 2 * K], BF16, tag="idxw")
            nc.sync.dma_start(idxw[:], idxw_d[:])
            blobr = constp.tile([P, BR_W], BF16, tag="blobr")
            nc.sync.dma_start(blobr[:], blobr_d[:])
            sel = constp.tile([NB, NRT, P], BF16, tag="sel")
            nc.sync.dma_start(sel[:], sel_d[:])
            xtb = constp.tile([P, DC, T], BF16, tag="xtb")
            for hh in range(4):
                h2 = ts(hh, DC // 4)
                nc.sync.dma_start(xtb[:, h2, :], xtb_d[:, h2, :])

            blobf = constp.tile([P, BF_W], F32, tag="blobf")
            nc.scalar.dma_start(blobf[:], blobf_d[:])
            a1 = constp.tile([P, DC, NB * R], F8, tag="a1f8")
            for hh in range(2):
                h2 = ts(hh, DC // 4)
                nc.scalar.dma_start(a1[:, h2, :], a1_d[:, h2, :])
            # exp of the recipe table early (needs only blobf)
            rec = blobf[:NN, BF_REC:BF_REC + NB]
            recsb = constp.tile([NN, NB], BF16, tag="recsb")
            ssum = small.tile([NN, 1], F32, tag="ssum")
            nc.scalar.activation(recsb[:], rec, AF.Exp, accum_out=ssum[:])
            for hh in range(2, 4):
                h2 = ts(hh, DC // 4)
                nc.scalar.dma_start(a1[:, h2, :], a1_d[:, h2, :])
            a2 = constp.tile([P, 2, 2, DT, P], F8, tag="a2")
            nc.scalar.dma_start(a2[:], a2_d[:])
            # anchor read for the warm-up matmuls (prevents dead-code elim);
            # on Scalar so it cannot delay the Vector scatter or the PE
            warm_anchor = small.tile([P, 1], F32, tag="warm_anchor")

            bu = blobf[:, BF_BU:BF_BU + FT]
            bd = blobf[:, BF_BD:BF_BD + DT]
            trep = blobr[:R, BR_TREP:BR_TREP + P]
            qred = blobr[:, BR_QRED:BR_QRED + R]
            identb = blobr[:, BR_ID:BR_ID + P]
            ident32 = blobr[:, BR_ID32:BR_ID32 + P]

            # ---- PE warm-up on the memset tile: starts at user-code time
            # zero with no DMA dependency so the clock ramp begins before the
            # first x/a1 chunk lands ----
            warm_ps = psumB.tile([P, T], F32, tag="b", name="warm")
            for w in range(NWARM):
                nc.tensor.matmul(warm_ps[:], warm_sb[:, :P], warm_sb[:],
                                 start=(w == 0), stop=(w == NWARM - 1))
            nc.scalar.activation(warm_anchor[:], warm_ps[:, 0:1], AF.Copy)

            # ---- routing scatter S[t, neuron] (weighted one-hot): one fused
            # 3-op Vector chain over all four token-tiles ----
            st_sb = constp.tile([NN, T], BF16, tag="st")
            iota4 = i512[:].rearrange(
                "p (o n k) -> p o n k", o=1, k=K).to_broadcast((P, TT, NN, K))
            idx_b = idxw[:, :, 0:K].rearrange(
                "p t (o k) -> p t o k", o=1).to_broadcast((P, TT, NN, K))
            w_b = idxw[:, :, K:2 * K].rearrange(
                "p t (o k) -> p t o k", o=1).to_broadcast((P, TT, NN, K))
            sk = smv.tile([P, TT, NN, K], BF16, tag="sk")
            s_all = smv.tile([P, TT, NN], BF16, tag="s")
            nc.vector.tensor_tensor(sk[:], iota4, idx_b, ALU.is_equal)
            nc.vector.tensor_tensor(sk[:], sk[:], w_b, ALU.mult)
            # pairwise-tree reduction over k (faster than reduce_sum on DVE)
            nc.vector.tensor_tensor(sk[:, :, :, 0:4], sk[:, :, :, 0:4],
                                    sk[:, :, :, 4:8], ALU.add)
            nc.vector.tensor_tensor(sk[:, :, :, 0:2], sk[:, :, :, 0:2],
                                    sk[:, :, :, 2:4], ALU.add)
            nc.vector.tensor_tensor(
                s_all[:].rearrange("p t (n o) -> p t n o", o=1),
                sk[:, :, :, 0:1], sk[:, :, :, 1:2], ALU.add)
            rsum = small.tile([NN, 1], F32, tag="rsum")
            nc.vector.reciprocal(rsum[:], ssum[:])

            yt_ps = [psum.tile([P, T], F32, tag="ps", name=f"yt{i}")
                     for i in range(NRT)]

            def yt_chunk(dc):
                for i in range(NRT):
                    nc.tensor.matmul(yt_ps[i][:], a1[:, dc, ts(i, P)],
                                     xtb[:, dc, :],
                                     start=(dc == 0), stop=(dc == DC - 1))

            # ---- YT chunks follow the x/a1 DMA; routing matmuls fill the
            # gaps between chunks ----
            yt_chunk(0)

            # recsT = recs^T, then M_i = (recs @ SEL_i)^T as [NN, P]
            rT_ps = psumB.tile([NB, NN], BF16, tag="b", name="rTps")
            nc.tensor.transpose(rT_ps[:], recsb[:], identb[:NN, :NN])
            recsT = constp.tile([NB, NN], BF16, tag="recsT")
            nc.scalar.activation(recsT[:], rT_ps[:], AF.Copy)

            yt_chunk(1)

            m_sb = []
            for i in range(NRT):
                mp = psumA.tile([NN, P], F32, tag="rp", name=f"m{i}")
                nc.tensor.matmul(mp[:], recsT[:], sel[:, i, :],
                                 start=True, stop=True)
                ms = constp.tile([NN, P], BF16, tag=f"m{i}", name=f"ms{i}")
                nc.scalar.activation(ms[:], mp[:], AF.Copy)
                m_sb.append(ms)

            yt_chunk(2)
            yt_chunk(3)
            yt_chunk(4)

            # routing transposes: all four into one PSUM tile, one drain
            stp_all = psumB.tile([NN, TT * P], BF16, tag="b", name="stp")
            for tt in range(TT):
                nc.tensor.transpose(stp_all[:, ts(tt, P)], s_all[:, tt, :],
                                    identb)
            nc.scalar.activation(st_sb[:], stp_all[:], AF.Copy,
                                 scale=rsum[:, 0:1])

            yt_chunk(5)

            # RepR[i][(n,r), t] = tr[t, n(i,p)] = M_i^T @ S^T, into one tile
            reprall = constp.tile([P, NRT, T], BF16, tag="reprall")
            rr_ps = [psumA.tile([P, T], F32, tag="rp", name=f"rp{i}")
                     for i in range(NRT)]

            def repr_block(i):
                nc.tensor.matmul(rr_ps[i][:], m_sb[i][:], st_sb[:],
                                 start=True, stop=True)
                if i % 2 == 0:
                    nc.vector.tensor_copy(reprall[:, i, :], rr_ps[i][:])
                else:
                    nc.scalar.activation(reprall[:, i, :], rr_ps[i][:], AF.Copy)

            yt_chunk(6)
            repr_block(0)
            repr_block(1)
            yt_chunk(7)
            repr_block(2)
            repr_block(3)

            # ---- WYT = YT * RepR;  hT = sum_n WYT ----
            ht_ps = psumB.tile([R, T], F32, tag="b", name="htps")
            wyt = constp.tile([P, NRT, T], BF16, tag="wyt")
            for i in range(NRT):
                nc.vector.tensor_mul(out=wyt[:, i, :], in0=yt_ps[i][:],
                                     in1=reprall[:, i, :])
                nc.tensor.matmul(ht_ps[:], qred, wyt[:, i, :],
                                 start=(i == 0), stop=(i == NRT - 1))
            ht_sb = constp.tile([R, T], BF16, tag="ht")
            nc.scalar.activation(ht_sb[:], ht_ps[:], AF.Copy)

            # ---- preload 32*x into PSUM (so the xf drain is one scaled
            # copy); these identity matmuls fill PE gaps while Vector builds
            # ct ----
            xf = constp.tile([P, DC, T], BF16, tag="a1f8", name="xf")
            dl_ps = {}
            for dt in range(2):
                dl_ps[dt] = psum.tile([P, T], F32, tag="ps", name=f"dl{dt}")
                nc.tensor.matmul(dl_ps[dt][:], ident32, xtb[:, dt, :],
                                 start=True, stop=False)

            # CT = RepH * RepR in fp8 (two halves so the first DoubleRow
            # pair can start early); RepH = trep @ hT
            rh_ps = psumA.tile([P, T], F32, tag="rp", name="rh")
            nc.tensor.matmul(rh_ps[:], trep, ht_sb[:], start=True, stop=True)
            for dt in range(2, 5):
                dl_ps[dt] = psum.tile([P, T], F32, tag="ps", name=f"dl{dt}")
                nc.tensor.matmul(dl_ps[dt][:], ident32, xtb[:, dt, :],
                                 start=True, stop=False)
            ct = constp.tile([P, NRT, T], F8, tag="ct")
            for i in range(NRT):
                nc.vector.tensor_tensor(ct[:, i, :], rh_ps[:],
                                        reprall[:, i, :], ALU.mult)

            # ---- deltaT: fp8 DoubleRow, dt-outer so drains chase;  xf =
            # (32*x + 32*alpha*delta) / 32 on alternating Scalar/Vector ----
            def drain_xf(dt):
                if dt % 2 == 0:
                    nc.scalar.activation(xf[:, dt, :], dl_ps[dt][:], AF.Copy,
                                         scale=1.0 / A2S)
                else:
                    nc.vector.tensor_scalar_mul(xf[:, dt, :], dl_ps[dt][:],
                                                1.0 / A2S)

            def delta_block(dt):
                for pi in range(2):
                    nc.tensor.matmul(dl_ps[dt][:], a2[:, pi, :, dt, :],
                                     ct[:, 2 * pi:2 * pi + 2, :],
                                     start=False, stop=(pi == 1),
                                     perf_mode=DR)
                drain_xf(dt)

            for dt in range(3):
                delta_block(dt)
            for dt in range(5, DT):
                dl_ps[dt] = psum.tile([P, T], F32, tag="ps", name=f"dl{dt}")
                nc.tensor.matmul(dl_ps[dt][:], ident32, xtb[:, dt, :],
                                 start=True, stop=False)
                delta_block(dt - 2)
            delta_block(6)
            delta_block(7)

            # prefetch the first two wd tiles now (on the sync ring, which is
            # idle after x) so the down projection never waits on DMA at the
            # up->down boundary
            wd_tiles = {}
            for dt in range(2):
                for h in range(2):
                    wdt = wdstream.tile([P, FT // 2, P], BF16, tag="wd",
                                        name=f"wd{dt}_{h}")
                    nc.sync.dma_start(wdt[:], wd_d[dt * 2 + h])
                    wd_tiles[(dt, h)] = wdt

            # ---- FFN up + exact gelu; wu streams on the scalar ring so it
            # never competes with x/consts on the sync ring ----
            g = constp.tile([P, FT, T], BF16, tag="g")
            for ftp in range(FT // 2):
                wu = stream.tile([P, 2, DC, P], BF16, tag="wu", name=f"wu{ftp}")
                nc.scalar.dma_start(wu[:], wu_d[ftp])
                for j in range(2):
                    ft = 2 * ftp + j
                    u_ps = psum.tile([P, T], F32, tag="ps", name=f"u{ft}")
                    for dc in range(DC):
                        nc.tensor.matmul(u_ps[:], wu[:, j, dc, :], xf[:, dc, :],
                                         start=(dc == 0), stop=(dc == DC - 1))
                    nc.scalar.activation(g[:, ft, :], u_ps[:], AF.Gelu,
                                         bias=bu[:, ft:ft + 1], scale=1.0)

            # ---- FFN down + bias; bias-add split Vector/Scalar and the
            # output DMA split across two queues to shorten the tail ----
            TH = T // 2
            for dt in range(DT):
                o_ps = psum.tile([P, T], F32, tag="ps", name=f"o{dt}")
                for h in range(2):
                    wd = wd_tiles.pop((dt, h))
                    for fc in range(FT // 2):
                        fcg = h * (FT // 2) + fc
                        nc.tensor.matmul(o_ps[:], wd[:, fc, :], g[:, fcg, :],
                                         start=(fcg == 0), stop=(fcg == FT - 1))
                    if dt + 2 < DT:
                        wdt = wdstream.tile([P, FT // 2, P], BF16, tag="wd",
                                            name=f"wd{dt + 2}_{h}")
                        nc.sync.dma_start(wdt[:], wd_d[(dt + 2) * 2 + h])
                        wd_tiles[(dt + 2, h)] = wdt
                otv = otp.tile([P, TH], BF16, tag="otv", name=f"otv{dt}")
                nc.vector.tensor_scalar_add(otv[:], o_ps[:, 0:TH],
                                            bd[:, dt:dt + 1])
                nc.sync.dma_start(out_d[:, dt, 0:TH], otv[:])
                otg = otp.tile([P, TH], BF16, tag="otg", name=f"otg{dt}")
                nc.scalar.activation(otg[:], o_ps[:, TH:T], AF.Identity,
                                     bias=bd[:, dt:dt + 1], scale=1.0)
                nc.scalar.dma_start(out_d[:, dt, TH:T], otg[:])

    nc.finalize()
    return nc


def _get_nc():
    if _BUILT[0] is None:
        _BUILT[0] = _build_nc()
    return _BUILT[0]


def kernel(x, neuron_idx, neuron_weights, neuron_recipe, basis_A,
           w_up_w, w_up_b, w_down_w, w_down_b, alpha):
    import ml_dtypes
    nc = _get_nc()

    x = np.asarray(x, dtype=np.float32).reshape(NCORES * T, D)
    idxf = np.asarray(neuron_idx).astype(np.float32).reshape(NCORES * T, K)
    wgt = np.asarray(neuron_weights, dtype=np.float32).reshape(NCORES * T, K)
    rec = np.asarray(neuron_recipe, dtype=np.float32)
    bA = np.asarray(basis_A, dtype=np.float32)
    wu = np.asarray(w_up_w, dtype=np.float32)
    bu_in = np.asarray(w_up_b, dtype=np.float32)
    wd = np.asarray(w_down_w, dtype=np.float32)
    bd_in = np.asarray(w_down_b, dtype=np.float32)
    alpha_f = float(np.asarray(alpha, dtype=np.float32))

    # replicated operands, packed into the on-device layouts
    # A1 in fp8, scaled by 16 to stay in e4m3 normal range (1/16 in qred)
    a1 = np.ascontiguousarray(
        np.clip(bA.transpose(1, 0, 2).reshape(D, NB * R) * A1S, -240.0, 240.0)
        .reshape(DC, P, NB * R).transpose(1, 0, 2)
    ).astype(ml_dtypes.float8_e4m3)
    # delta projection in fp8: 32*alpha*A2, [(pair, j, p), (dt, c)] packed
    a2m = np.clip(bA.transpose(0, 2, 1).reshape(NB * R, D) * (alpha_f * A2S),
                  -240.0, 240.0)
    a2 = np.ascontiguousarray(
        a2m.reshape(2, 2, P, DT, P).transpose(2, 0, 1, 3, 4)
    ).astype(ml_dtypes.float8_e4m3)
    wu_p = np.ascontiguousarray(
        wu.reshape(DC, P, FT // 2, 2, P).transpose(2, 1, 3, 0, 4)
    ).astype(ml_dtypes.bfloat16)
    wd_p = np.ascontiguousarray(
        wd.reshape(2, FT // 2, P, DT, P).transpose(3, 0, 2, 1, 4)
        .reshape(DT * 2, P, FT // 2, P)).astype(ml_dtypes.bfloat16)

    blobf = np.zeros((P, BF_W), dtype=np.float32)
    blobf[:, BF_BU:BF_BU + FT] = bu_in.reshape(FT, P).T
    blobf[:, BF_BD:BF_BD + DT] = bd_in.reshape(DT, P).T
    blobf[:NN, BF_REC:BF_REC + NB] = rec

    blobr = np.zeros((P, BR_W), dtype=np.float32)
    blobr[:R, BR_TREP:BR_TREP + P] = (
        np.arange(P)[None, :] % R == np.arange(R)[:, None])
    blobr[:, BR_QRED:BR_QRED + R] = (
        np.arange(P)[:, None] % R == np.arange(R)[None, :]) / A1S
    blobr[:, BR_ID:BR_ID + P] = np.eye(P, dtype=np.float32)
    blobr[:, BR_ID32:BR_ID32 + P] = np.eye(P, dtype=np.float32) * A2S
    blobr = blobr.astype(ml_dtypes.bfloat16)

    # SEL[n, i, m] = 1 iff n in [4i, 4i+4) and m // 32 == n - 4i
    sel = np.zeros((NB, NRT, P), dtype=np.float32)
    for n in range(NB):
        i, nloc = divmod(n, NRT)
        sel[n, i, nloc * R:(nloc + 1) * R] = 1.0
    sel = sel.astype(ml_dtypes.bfloat16)

    shared = {
        "blobf": blobf, "blobr": blobr, "sel": sel, "a1": a1, "a2": a2,
        "wu": wu_p, "wd": wd_p,
    }
    in_maps = []
    idxw = np.concatenate([idxf, wgt], axis=1).astype(
        ml_dtypes.bfloat16)  # [N*T, 16]
    for c in range(NCORES):
        xc = x[c * T:(c + 1) * T]  # [T, D]
        xtc = np.ascontiguousarray(xc.T.reshape(DC, P, T).transpose(1, 0, 2))
        xtbc = xtc.astype(ml_dtypes.bfloat16)
        iwc = np.ascontiguousarray(
            idxw[c * T:(c + 1) * T].reshape(TT, P, 2 * K).transpose(1, 0, 2))
        in_maps.append({"xtb": xtbc, "idxw": iwc, **shared})

    res = run_bass_kernel_spmd(nc, in_maps, core_ids=list(range(NCORES)))

    out = np.empty((NCORES * T, D), dtype=np.float32)
    for c in range(NCORES):
        ot = res.results[c]["outT"].astype(np.float32)  # [P, DT, T]
        out[c * T:(c + 1) * T] = ot.transpose(1, 0, 2).reshape(D, T).T
    return out.reshape(2, 2048, D)
